# revision 32
# baseline (speedup 1.0000x reference)
"""Trainium2 Bass kernel for a 4-layer BYO-GPT dense transformer.

Contract: kernel(**inputs) takes the FULL unsharded inputs (as produced by
reference.setup_inputs()) and returns the FULL logits [B, S, VOCAB] fp32.

Sharding (8 cores, no collectives):
  core c: batch b = c // 4, vocab shard s = c % 4.
  - Each core runs the full 4-layer transformer for its batch element
    (data-parallel over B=2, replicated 4x within the batch group).
  - The unembed is sharded row-wise over vocab: vocab padded 50257 -> 50688,
    each core computes logits for its 12672-row shard; the host concatenates.
  - Embedding rows are gathered host-side per batch (input sharding: each core
    receives exactly the embed rows for its tokens); +PE and everything else
    runs on device.

Device program (per core), activations natural [t, d] in SBUF:
  - per layer: PE-transpose x -> x_T; qkv/linear use host-pre-transposed
    weights so every matmul is a direct lhsT.T @ rhs with K=d on partitions.
  - attention: scores transposed [j, i] = K_h^T.T-matmul, causal mask via a
    constant [128,128] tile on the diagonal block, exp without max-subtraction
    (scores are O(1) here). P.V computed transposed ([65, i], wide-N matmuls
    with a ones-column in the V tile producing the softmax denominator in the
    same matmul), then PE-transposed back to natural [i, 65] and normalized
    with a per-partition reciprocal broadcast.
  - matmul dtypes: float32r (full-rate fp32; producers must be f32r-typed for
    the BIR verifier) for projections/linear, bf16 for scores and P.V (small-N
    matmuls where fp32r is 4x slower) and for the unembed weights/x4_T (halves
    the unembed weight DMA, which otherwise starves the PE ~9us per v-tile).
"""

import os
import sys

for _p in ("/opt/trn_rl_repo", "/root/.axon_site", "/root/.axon_site/_ro/trn_rl_repo",
           "/root/.axon_site/_ro/pypackages"):
    if os.path.isdir(_p) and _p not in sys.path:
        sys.path.append(_p)

import numpy as np

import concourse.bass as bass
import concourse.mybir as mybir
import concourse.tile as tile
from concourse import bacc
from concourse.bass_utils import run_bass_kernel_spmd
from concourse.masks import make_identity

F32 = mybir.dt.float32
F32R = mybir.dt.float32r
BF16 = mybir.dt.bfloat16
AF = mybir.ActivationFunctionType
ALU = mybir.AluOpType

P = 128
D = 768
KC = D // P          # 6 d-chunks
NHEAD = 12
DH = 64
NPAIR = 6            # head pairs (2 heads / 128 partitions)
EPS = 1e-5
VOCAB = 50257
B = 2
S = 1024
NCORES = 8
VPAD = 50688         # 396 * 128, divisible by 4
VSH = VPAD // 4      # 12672 per-core vocab shard
NEG = -1.0e30


def _bcast(ap_1d, p=P):
    """Partition-broadcast AP: replicate a 1-D (or row) DRAM AP across p partitions."""
    return bass.AP(tensor=ap_1d.tensor, offset=ap_1d.offset,
                   ap=[[0, p]] + [list(x) for x in ap_1d.ap])


def _r(x):
    return x.bitcast(F32R)


BUILD_VER = 12  # bump on every program change: the axon terminal caches
               # executables without hashing the embedded BIR, so the HLO
               # must differ structurally (vtag input shape) per version.


def build_program(S_c=S, L=4, VSH_c=VSH, dbg=False):
    """Build the per-core Bass program. Returns compiled nc."""
    TC = S_c // P        # token chunks
    nc = bacc.Bacc("TRN2", target_bir_lowering=False, debug=False)

    # ---- DRAM I/O ----
    vtag = nc.dram_tensor("vtag", [1, BUILD_VER], F32, kind="ExternalInput")
    x0 = nc.dram_tensor("x0", [S_c, D], F32, kind="ExternalInput")
    pe = nc.dram_tensor("pe", [S_c, D], F32, kind="ExternalInput")
    wqT = nc.dram_tensor("wqT", [L, D, D], F32R, kind="ExternalInput")
    wkT = nc.dram_tensor("wkT", [L, D, D], F32R, kind="ExternalInput")
    wvT = nc.dram_tensor("wvT", [L, D, D], F32R, kind="ExternalInput")
    wlT = nc.dram_tensor("wlT", [L, D, D], F32R, kind="ExternalInput")
    bq = nc.dram_tensor("bq", [L, D], F32, kind="ExternalInput")
    bk = nc.dram_tensor("bk", [L, D], F32, kind="ExternalInput")
    bv = nc.dram_tensor("bv", [L, D], F32, kind="ExternalInput")
    bl = nc.dram_tensor("bl", [L, D], F32, kind="ExternalInput")
    s1 = nc.dram_tensor("s1", [L, D], F32, kind="ExternalInput")
    b1 = nc.dram_tensor("b1", [L, D], F32, kind="ExternalInput")
    s2 = nc.dram_tensor("s2", [L, D], F32, kind="ExternalInput")
    b2 = nc.dram_tensor("b2", [L, D], F32, kind="ExternalInput")
    uT = nc.dram_tensor("uT", [D, VSH_c], BF16, kind="ExternalInput")
    ub = nc.dram_tensor("ub", [VSH_c], F32, kind="ExternalInput")
    mask = nc.dram_tensor("mask", [P, P], F32, kind="ExternalInput")
    logits = nc.dram_tensor("logits", [S_c, VSH_c], F32, kind="ExternalOutput")
    if dbg:
        dbg_x = nc.dram_tensor("dbg_x", [L + 1, S_c, D], F32, kind="ExternalOutput")
        dbg_attn = nc.dram_tensor("dbg_attn", [L, S_c, D], F32, kind="ExternalOutput")

    x0_t = x0.rearrange("(tc p) d -> p tc d", p=P)
    pe_t = pe.rearrange("(tc p) d -> p tc d", p=P)
    logits_t = logits.rearrange("(tc p) v -> p tc v", p=P)

    # P_T block offsets: scores for j-chunk jc cover i in [128*jc, S_c)
    W = [S_c - P * jc for jc in range(TC)]
    OFF = [0] * TC
    for jc in range(1, TC):
        OFF[jc] = OFF[jc - 1] + W[jc - 1]
    PTW = OFF[-1] + W[-1]

    with tile.TileContext(nc) as tc_:
        from contextlib import ExitStack
        with ExitStack() as ctx:
            # outer pools: live for the whole program
            const = ctx.enter_context(tc_.tile_pool(name="const", bufs=1))
            xpool = ctx.enter_context(tc_.tile_pool(name="xpool", bufs=1))
            xtpool = ctx.enter_context(tc_.tile_pool(name="xtpool", bufs=1))
            # layer-phase pools: freed before the unembed phase opens its own
            lctx = ctx.enter_context(ExitStack())
            ps512 = lctx.enter_context(tc_.tile_pool(name="ps512", bufs=4, space="PSUM"))
            anpool = lctx.enter_context(tc_.tile_pool(name="anpool", bufs=1))
            wfull = lctx.enter_context(tc_.tile_pool(name="wfull", bufs=2))
            wqk = lctx.enter_context(tc_.tile_pool(name="wqk", bufs=3))
            qkp = lctx.enter_context(tc_.tile_pool(name="qkp", bufs=6))
            vap = lctx.enter_context(tc_.tile_pool(name="vap", bufs=1))
            ptp = lctx.enter_context(tc_.tile_pool(name="ptp", bufs=3))
            lnp = lctx.enter_context(tc_.tile_pool(name="lnp", bufs=1))
            ztp = lctx.enter_context(tc_.tile_pool(name="ztp", bufs=2))
            stp = lctx.enter_context(tc_.tile_pool(name="stp", bufs=6))
            biasp = lctx.enter_context(tc_.tile_pool(name="biasp", bufs=2))
            pet = lctx.enter_context(tc_.tile_pool(name="pet", bufs=2))
            psatt = lctx.enter_context(tc_.tile_pool(name="psatt", bufs=2, space="PSUM"))
            pstr = lctx.enter_context(tc_.tile_pool(name="pstr", bufs=2, space="PSUM"))

            ident = const.tile([P, P], F32)
            make_identity(nc, ident)
            mask_s = const.tile([P, P], F32)
            nc.sync.dma_start(mask_s[:], mask[:])
            eps_t = const.tile([P, 1], F32)
            nc.vector.memset(eps_t[:], EPS)
            vt_t = const.tile([1, BUILD_VER], F32)
            nc.sync.dma_start(vt_t[:], vtag[:])

            # ---- embedding: x = x0 + pe ----
            # per-chunk tiles so Tile's dependency tracking stays fine-grained:
            # readers of chunk t must not wait on writers of other chunks
            xs = [xpool.tile([P, D], F32, tag=f"x{t}", name=f"x{t}") for t in range(TC)]
            for t in range(TC):
                nc.sync.dma_start(xs[t][:], x0_t[:, t, :])
                pe_c = pet.tile([P, D], F32)
                nc.sync.dma_start(pe_c[:], pe_t[:, t, :])
                nc.vector.tensor_add(xs[t][:], xs[t][:], pe_c[:])

            def transpose_x(dst_xt):
                """PE-transpose x_nat [t,d] -> dst_xt [128, KC, S_c] ([d,t])."""
                for t in range(TC):
                    for k in range(KC):
                        pt = pstr.tile([P, P], F32)
                        nc.tensor.transpose(pt[:], xs[t][:, k * P:(k + 1) * P], ident[:])
                        nc.any.tensor_copy(dst_xt[:, k, t * P:(t + 1) * P], pt[:])

            def layernorm_chunk(xc, s_b, b_b, eng):
                """In-place LN over free dim (768) of xc [128, 768]."""
                stats = stp.tile([P, 3, 6], F32)
                for g in range(3):
                    nc.vector.bn_stats(stats[:, g, :], xc[:, g * 256:(g + 1) * 256])
                mv = stp.tile([P, 2], F32)
                nc.vector.bn_aggr(mv[:], stats[:])
                sd = stp.tile([P, 1], F32)
                nc.scalar.activation(sd[:], mv[:, 1:2], AF.Sqrt, bias=eps_t[:], scale=1.0)
                rs = stp.tile([P, 1], F32)
                nc.vector.reciprocal(rs[:], sd[:])
                nm = stp.tile([P, 1], F32)
                nc.vector.tensor_mul(nm[:], mv[:, 0:1], rs[:])
                nc.vector.tensor_scalar_mul(nm[:], nm[:], -1.0)
                nc.scalar.activation(xc, xc, AF.Identity, bias=nm[:], scale=rs[:])
                nc.vector.tensor_mul(xc, xc, s_b)
                eng.tensor_add(xc, xc, b_b)

            def dbg_dump(dst, idx, tiles):
                d_t = dst.rearrange("l (tc p) d -> l p tc d", p=P)
                for t in range(TC):
                    nc.sync.dma_start(d_t[idx, :, t, :], tiles[t][:])

            if dbg:
                dbg_dump(dbg_x, 0, xs)

            for l in range(L):
                # per-layer params (partition-broadcast replicas / per-o tiles)
                s1b = lnp.tile([P, D], F32, tag="s1b")
                b1b = lnp.tile([P, D], F32, tag="b1b")
                s2b = lnp.tile([P, D], F32, tag="s2b")
                b2b = lnp.tile([P, D], F32, tag="b2b")
                bvb = lnp.tile([P, D], F32, tag="bvb")
                blb = lnp.tile([P, D], F32, tag="blb")
                for t_, src in ((s1b, s1), (b1b, b1), (s2b, s2), (b2b, b2), (bvb, bv), (blb, bl)):
                    nc.gpsimd.dma_start(t_[:], _bcast(src[l]))
                bq_t = biasp.tile([P, NPAIR], F32, tag="bq")
                bk_t = biasp.tile([P, NPAIR], F32, tag="bk")
                nc.sync.dma_start(bq_t[:], bq[l].rearrange("(c p) -> p c", p=P))
                nc.sync.dma_start(bk_t[:], bk[l].rearrange("(c p) -> p c", p=P))

                x_T = xtpool.tile([P, KC, S_c], F32R, tag="xT")
                transpose_x(x_T)

                # ---- v projection (all heads at once, N>=256) ----
                wv_s = wfull.tile([P, KC, D], F32R, tag="wbig")
                nc.sync.dma_start(wv_s[:], wvT[l].rearrange("(k p) o -> p k o", p=P))
                v_aug = vap.tile([P, TC, NHEAD, DH + 1], BF16)
                nc.vector.memset(v_aug[:, :, :, DH:DH + 1], 1.0)
                for t in range(TC):
                    for os_, ow in ((0, 512), (512, 256)):
                        pv = ps512.tile([P, 512], F32, tag="ps512")
                        for k in range(KC):
                            nc.tensor.matmul(pv[:, :ow], x_T[:, k, t * P:(t + 1) * P],
                                             wv_s[:, k, os_:os_ + ow],
                                             start=(k == 0), stop=(k == KC - 1))
                        nh0 = os_ // DH
                        nc.vector.tensor_tensor(
                            v_aug[:, t, nh0:nh0 + ow // DH, 0:DH],
                            pv[:, :ow].rearrange("p (h d) -> p h d", d=DH),
                            bvb[:, os_:os_ + ow].rearrange("p (h d) -> p h d", d=DH),
                            ALU.add)

                attn_nat = [anpool.tile([P, D], F32, tag=f"an{t}", name=f"an{t}") for t in range(TC)]

                for pr in range(NPAIR):
                    wq_s = wqk.tile([P, KC, P], F32R, tag="wqk")
                    wk_s = wqk.tile([P, KC, P], F32R, tag="wqk")
                    nc.sync.dma_start(wq_s[:], wqT[l].rearrange("(k p) o -> p k o", p=P)[:, :, pr * P:(pr + 1) * P])
                    nc.sync.dma_start(wk_s[:], wkT[l].rearrange("(k p) o -> p k o", p=P)[:, :, pr * P:(pr + 1) * P])
                    qT_p = qkp.tile([P, S_c], BF16, tag="qk")
                    kT_p = qkp.tile([P, S_c], BF16, tag="qk")
                    for dst, w_s, b_t in ((qT_p, wq_s, bq_t), (kT_p, wk_s, bk_t)):
                        for nt in range(0, S_c, 512):
                            nw = min(512, S_c - nt)
                            pq = ps512.tile([P, 512], F32, tag="ps512")
                            for k in range(KC):
                                nc.tensor.matmul(pq[:, :nw], w_s[:, k, :],
                                                 x_T[:, k, nt:nt + nw],
                                                 start=(k == 0), stop=(k == KC - 1))
                            nc.scalar.activation(dst[:, nt:nt + nw], pq[:, :nw],
                                                 AF.Identity, bias=b_t[:, pr:pr + 1], scale=1.0)

                    for hh in range(2):
                        h = 2 * pr + hh
                        hs = DH * hh
                        # scores (transposed [j, i]) -> exp -> P_T (bf16)
                        p_t = ptp.tile([P, PTW], BF16, tag="pt")
                        for jc in range(TC):
                            w_ = W[jc]
                            i0 = P * jc
                            for so in range(0, w_, 512):
                                sw = min(512, w_ - so)
                                ps = ps512.tile([P, 512], F32, tag="ps512")
                                nc.tensor.matmul(
                                    ps[:, :sw],
                                    kT_p[hs:hs + DH, i0:i0 + P],
                                    qT_p[hs:hs + DH, i0 + so:i0 + so + sw],
                                    start=True, stop=True)
                                if so == 0:
                                    nc.vector.tensor_tensor(ps[:, :P], ps[:, :P], mask_s[:], ALU.add)
                                nc.scalar.activation(p_t[:, OFF[jc] + so:OFF[jc] + so + sw],
                                                     ps[:, :sw], AF.Exp, scale=0.125)
                        # P.V transposed: attnT_aug [65, i] per 512-wide i-tile
                        # (row 64 = softmax denominator), then PE-transpose back
                        # to natural [i, 65] and normalize.
                        for it in range((S_c + 511) // 512):
                            i_lo = 512 * it
                            i_hi = min(512 * (it + 1), S_c)
                            jcmax = min(TC - 1, (i_hi - 1) // P)
                            pat = psatt.tile([P, 512], F32, tag="pat")
                            for jc in range(jcmax + 1):
                                s0 = max(i_lo, P * jc)
                                w_ = i_hi - s0
                                o_ = OFF[jc] + s0 - P * jc
                                nc.tensor.matmul(
                                    pat[0:DH + 1, s0 - i_lo:s0 - i_lo + w_],
                                    v_aug[:, jc, h, :],
                                    p_t[:, o_:o_ + w_],
                                    start=(jc == 0), stop=(jc == jcmax))
                            atT = ztp.tile([DH + 1, 512], F32, tag="atT")
                            nc.any.tensor_copy(atT[:, :i_hi - i_lo], pat[0:DH + 1, :i_hi - i_lo])
                            for k_ in range((i_hi - i_lo) // P):
                                ic = 4 * it + k_
                                pa2 = pstr.tile([P, P], F32, tag="pt")
                                nc.tensor.transpose(pa2[:, 0:DH + 1], atT[:, k_ * P:(k_ + 1) * P],
                                                    ident[0:DH + 1, 0:DH + 1])
                                r_ = stp.tile([P, 1], F32)
                                nc.vector.reciprocal(r_[:], pa2[:, DH:DH + 1])
                                nc.vector.tensor_mul(attn_nat[ic][:, h * DH:(h + 1) * DH],
                                                     pa2[:, 0:DH],
                                                     r_[:].to_broadcast((P, DH)))

                if dbg:
                    dbg_dump(dbg_attn, l, attn_nat)

                # ---- residual + LN1 ----
                for t in range(TC):
                    eng = nc.gpsimd
                    eng.tensor_add(xs[t][:], xs[t][:], attn_nat[t][:])
                    layernorm_chunk(xs[t][:], s1b[:], b1b[:], eng)

                # ---- linear + residual + LN2 ----
                x1_T = xtpool.tile([P, KC, S_c], F32R, tag="xT")
                transpose_x(x1_T)
                wl_s = wfull.tile([P, KC, D], F32R, tag="wbig")
                nc.sync.dma_start(wl_s[:], wlT[l].rearrange("(k p) o -> p k o", p=P))
                for t in range(TC):
                    zt = ztp.tile([P, D], F32, tag="zt")
                    for os_, ow in ((0, 512), (512, 256)):
                        pl_ = ps512.tile([P, 512], F32, tag="ps512")
                        for k in range(KC):
                            nc.tensor.matmul(pl_[:, :ow], x1_T[:, k, t * P:(t + 1) * P],
                                             wl_s[:, k, os_:os_ + ow],
                                             start=(k == 0), stop=(k == KC - 1))
                        nc.vector.tensor_tensor(zt[:, os_:os_ + ow], pl_[:, :ow],
                                                blb[:, os_:os_ + ow], ALU.add)
                    eng = nc.gpsimd
                    eng.tensor_add(xs[t][:], xs[t][:], zt[:])
                    layernorm_chunk(xs[t][:], s2b[:], b2b[:], eng)

                if dbg:
                    dbg_dump(dbg_x, l + 1, xs)

            # ---- unembed ----
            x4_T = xtpool.tile([P, KC, S_c], BF16, tag="xT")
            transpose_x(x4_T)
            lctx.close()  # free layer-phase SBUF/PSUM before unembed pools
            psu = ctx.enter_context(tc_.tile_pool(name="psu", bufs=6, space="PSUM"))
            upool = ctx.enter_context(tc_.tile_pool(name="upool", bufs=4))
            ubp = ctx.enter_context(tc_.tile_pool(name="ubp", bufs=2))
            lop = ctx.enter_context(tc_.tile_pool(name="lop", bufs=3))
            uT_t = uT.rearrange("(k p) v -> p k v", p=P)
            for vs in range(0, VSH_c, 512):
                vw = min(512, VSH_c - vs)
                u_s = upool.tile([P, KC, 512], BF16, tag="u")
                nc.sync.dma_start(u_s[:, :, :vw], uT_t[:, :, vs:vs + vw])
                ub_b = ubp.tile([P, 512], F32, tag="ubb")
                nc.gpsimd.dma_start(ub_b[:, :vw], _bcast(ub[vs:vs + vw]))
                for t in range(TC):
                    pu = psu.tile([P, 512], F32, tag="psu")
                    for k in range(KC):
                        nc.tensor.matmul(pu[:, :vw], x4_T[:, k, t * P:(t + 1) * P],
                                         u_s[:, k, :vw],
                                         start=(k == 0), stop=(k == KC - 1))
                    lo = lop.tile([P, 512], F32, tag="lo")
                    nc.vector.tensor_tensor(lo[:, :vw], pu[:, :vw], ub_b[:, :vw], ALU.add)
                    nc.sync.dma_start(logits_t[:, t, vs:vs + vw], lo[:, :vw])

    nc.compile()
    return nc


_CACHE = {}


def get_program(S_c=S, L=4, VSH_c=VSH, dbg=False):
    key = (S_c, L, VSH_c, dbg)
    if key not in _CACHE:
        _CACHE[key] = build_program(S_c, L, VSH_c, dbg)
    return _CACHE[key]


def make_mask():
    jl = np.arange(P)[:, None]
    il = np.arange(P)[None, :]
    return np.where(jl <= il, 0.0, NEG).astype(np.float32)


def make_core_inputs(tokens, embed, pe, wq_w, wq_b, wk_w, wk_b, wv_w, wv_b,
                     lin_w, lin_b, n1_s, n1_b, n2_s, n2_b, unembed_w, unembed_b,
                     S_c=S, L=4, VSH_c=VSH, n_vshard=4):
    """Host-side sharding: returns list of in_maps (one per core)."""
    c = np.ascontiguousarray
    f = np.float32
    tokens = np.asarray(tokens)
    embed = np.asarray(embed, f)
    pe_s = c(np.asarray(pe, f)[:S_c])
    wqT = c(np.asarray(wq_w, f)[:L].transpose(0, 2, 1))
    wkT = c(np.asarray(wk_w, f)[:L].transpose(0, 2, 1))
    wvT = c(np.asarray(wv_w, f)[:L].transpose(0, 2, 1))
    wlT = c(np.asarray(lin_w, f)[:L].transpose(0, 2, 1))
    upad = np.zeros((n_vshard * VSH_c, D), f)
    ubpad = np.zeros((n_vshard * VSH_c,), f)
    nv = min(VOCAB, n_vshard * VSH_c, np.asarray(unembed_w).shape[0])
    upad[:nv] = np.asarray(unembed_w, f)[:nv]
    ubpad[:nv] = np.asarray(unembed_b, f)[:nv]
    mask = make_mask()
    common = dict(vtag=np.zeros((1, BUILD_VER), f), pe=pe_s, wqT=wqT, wkT=wkT, wvT=wvT, wlT=wlT,
                  bq=c(np.asarray(wq_b, f)[:L]), bk=c(np.asarray(wk_b, f)[:L]),
                  bv=c(np.asarray(wv_b, f)[:L]), bl=c(np.asarray(lin_b, f)[:L]),
                  s1=c(np.asarray(n1_s, f)[:L]), b1=c(np.asarray(n1_b, f)[:L]),
                  s2=c(np.asarray(n2_s, f)[:L]), b2=c(np.asarray(n2_b, f)[:L]),
                  mask=mask)
    n_batch_groups = NCORES // n_vshard
    in_maps = []
    for core in range(NCORES):
        b = core // n_vshard
        s_ = core % n_vshard
        x0 = c(embed[tokens[b, :S_c]])
        import ml_dtypes
        uT_c = c(upad[s_ * VSH_c:(s_ + 1) * VSH_c].T.astype(ml_dtypes.bfloat16))
        in_maps.append(dict(common, x0=x0, uT=uT_c,
                            ub=c(ubpad[s_ * VSH_c:(s_ + 1) * VSH_c])))
    return in_maps


def kernel(**inputs):
    nc = get_program(S, 4, VSH, dbg=False)
    in_maps = make_core_inputs(**inputs)
    res = run_bass_kernel_spmd(nc, in_maps, core_ids=list(range(NCORES)))
    out = np.zeros((B, S, VOCAB), np.float32)
    for core in range(NCORES):
        b = core // 4
        s_ = core % 4
        lo = res.results[core]["logits"]
        v0 = s_ * VSH
        v1 = min(v0 + VSH, VOCAB)
        if v1 > v0:
            out[b, :, v0:v1] = lo[:, :v1 - v0]
    return out


# revision 34
# speedup vs baseline: 1.4517x; 1.4517x over previous
"""Trainium2 Bass kernel for a 4-layer BYO-GPT dense transformer.

Contract: kernel(**inputs) takes the FULL unsharded inputs (as produced by
reference.setup_inputs()) and returns the FULL logits [B, S, VOCAB] fp32.

Sharding (8 cores, no collectives):
  core c: batch b = c // 4, vocab shard s = c % 4.
  - Each core runs the full 4-layer transformer for its batch element
    (data-parallel over B=2, replicated 4x within the batch group).
  - The unembed is sharded row-wise over vocab: vocab padded 50257 -> 50688,
    each core computes logits for its 12672-row shard; the host concatenates.
  - Embedding rows are gathered host-side per batch (input sharding: each core
    receives exactly the embed rows for its tokens); +PE and everything else
    runs on device.

Device program (per core), activations natural [t, d] in SBUF:
  - per layer: PE-transpose x -> x_T; qkv/linear use host-pre-transposed
    weights so every matmul is a direct lhsT.T @ rhs with K=d on partitions.
  - attention: scores transposed [j, i] = K_h^T.T-matmul, causal mask via a
    constant [128,128] tile on the diagonal block, exp without max-subtraction
    (scores are O(1) here). P.V computed transposed ([65, i], wide-N matmuls
    with a ones-column in the V tile producing the softmax denominator in the
    same matmul), then PE-transposed back to natural [i, 65] and normalized
    with a per-partition reciprocal broadcast.
  - matmul dtypes: float32r (full-rate fp32; producers must be f32r-typed for
    the BIR verifier) for projections/linear, bf16 for scores and P.V (small-N
    matmuls where fp32r is 4x slower) and for the unembed weights/x4_T (halves
    the unembed weight DMA, which otherwise starves the PE ~9us per v-tile).
"""

import os
import sys

for _p in ("/opt/trn_rl_repo", "/root/.axon_site", "/root/.axon_site/_ro/trn_rl_repo",
           "/root/.axon_site/_ro/pypackages"):
    if os.path.isdir(_p) and _p not in sys.path:
        sys.path.append(_p)

import numpy as np

import concourse.bass as bass
import concourse.mybir as mybir
import concourse.tile as tile
from concourse import bacc
from concourse.bass_utils import run_bass_kernel_spmd
from concourse.masks import make_identity

F32 = mybir.dt.float32
F32R = mybir.dt.float32r
BF16 = mybir.dt.bfloat16
AF = mybir.ActivationFunctionType
ALU = mybir.AluOpType

P = 128
D = 768
KC = D // P          # 6 d-chunks
NHEAD = 12
DH = 64
NPAIR = 6            # head pairs (2 heads / 128 partitions)
EPS = 1e-5
VOCAB = 50257
B = 2
S = 1024
NCORES = 8
VPAD = 50688         # 396 * 128, divisible by 4
VSH = VPAD // 4      # 12672 per-core vocab shard
NEG = -1.0e30


def _bcast(ap_1d, p=P):
    """Partition-broadcast AP: replicate a 1-D (or row) DRAM AP across p partitions."""
    return bass.AP(tensor=ap_1d.tensor, offset=ap_1d.offset,
                   ap=[[0, p]] + [list(x) for x in ap_1d.ap])


def _r(x):
    return x.bitcast(F32R)


BUILD_VER = 13  # bump on every program change: the axon terminal caches
               # executables without hashing the embedded BIR, so the HLO
               # must differ structurally (vtag input shape) per version.


def build_program(S_c=S, L=4, VSH_c=VSH, dbg=False):
    """Build the per-core Bass program. Returns compiled nc."""
    TC = S_c // P        # token chunks
    nc = bacc.Bacc("TRN2", target_bir_lowering=False, debug=False)

    # ---- DRAM I/O ----
    vtag = nc.dram_tensor("vtag", [1, BUILD_VER], F32, kind="ExternalInput")
    x0 = nc.dram_tensor("x0", [S_c, D], F32, kind="ExternalInput")
    pe = nc.dram_tensor("pe", [S_c, D], F32, kind="ExternalInput")
    wqT = nc.dram_tensor("wqT", [L, D, D], F32R, kind="ExternalInput")
    wkT = nc.dram_tensor("wkT", [L, D, D], F32R, kind="ExternalInput")
    wvT = nc.dram_tensor("wvT", [L, D, D], F32R, kind="ExternalInput")
    wlT = nc.dram_tensor("wlT", [L, D, D], F32R, kind="ExternalInput")
    bq = nc.dram_tensor("bq", [L, D], F32, kind="ExternalInput")
    bk = nc.dram_tensor("bk", [L, D], F32, kind="ExternalInput")
    bv = nc.dram_tensor("bv", [L, D], F32, kind="ExternalInput")
    bl = nc.dram_tensor("bl", [L, D], F32, kind="ExternalInput")
    s1 = nc.dram_tensor("s1", [L, D], F32, kind="ExternalInput")
    b1 = nc.dram_tensor("b1", [L, D], F32, kind="ExternalInput")
    s2 = nc.dram_tensor("s2", [L, D], F32, kind="ExternalInput")
    b2 = nc.dram_tensor("b2", [L, D], F32, kind="ExternalInput")
    uT = nc.dram_tensor("uT", [D, VSH_c], BF16, kind="ExternalInput")
    ub = nc.dram_tensor("ub", [VSH_c], F32, kind="ExternalInput")
    mask = nc.dram_tensor("mask", [P, P], F32, kind="ExternalInput")
    logits = nc.dram_tensor("logits", [S_c, VSH_c], F32, kind="ExternalOutput")
    if dbg:
        dbg_x = nc.dram_tensor("dbg_x", [L + 1, S_c, D], F32, kind="ExternalOutput")
        dbg_attn = nc.dram_tensor("dbg_attn", [L, S_c, D], F32, kind="ExternalOutput")

    x0_t = x0.rearrange("(tc p) d -> p tc d", p=P)
    pe_t = pe.rearrange("(tc p) d -> p tc d", p=P)
    logits_t = logits.rearrange("(tc p) v -> p tc v", p=P)

    # P_T block offsets: scores for j-chunk jc cover i in [128*jc, S_c)
    W = [S_c - P * jc for jc in range(TC)]
    OFF = [0] * TC
    for jc in range(1, TC):
        OFF[jc] = OFF[jc - 1] + W[jc - 1]
    PTW = OFF[-1] + W[-1]

    with tile.TileContext(nc) as tc_:
        from contextlib import ExitStack
        with ExitStack() as ctx:
            # outer pools: live for the whole program
            const = ctx.enter_context(tc_.tile_pool(name="const", bufs=1))
            xpool = ctx.enter_context(tc_.tile_pool(name="xpool", bufs=1))
            xtpool = ctx.enter_context(tc_.tile_pool(name="xtpool", bufs=1))
            # layer-phase pools: freed before the unembed phase opens its own
            lctx = ctx.enter_context(ExitStack())
            ps512 = lctx.enter_context(tc_.tile_pool(name="ps512", bufs=4, space="PSUM"))
            anpool = lctx.enter_context(tc_.tile_pool(name="anpool", bufs=1))
            wfull = lctx.enter_context(tc_.tile_pool(name="wfull", bufs=2))
            wqk = lctx.enter_context(tc_.tile_pool(name="wqk", bufs=3))
            qkp = lctx.enter_context(tc_.tile_pool(name="qkp", bufs=6))
            vap = lctx.enter_context(tc_.tile_pool(name="vap", bufs=1))
            ptp = lctx.enter_context(tc_.tile_pool(name="ptp", bufs=3))
            lnp = lctx.enter_context(tc_.tile_pool(name="lnp", bufs=1))
            ztp = lctx.enter_context(tc_.tile_pool(name="ztp", bufs=2))
            stp = lctx.enter_context(tc_.tile_pool(name="stp", bufs=6))
            biasp = lctx.enter_context(tc_.tile_pool(name="biasp", bufs=2))
            pet = lctx.enter_context(tc_.tile_pool(name="pet", bufs=2))
            psatt = lctx.enter_context(tc_.tile_pool(name="psatt", bufs=2, space="PSUM"))
            pstr = lctx.enter_context(tc_.tile_pool(name="pstr", bufs=2, space="PSUM"))

            ident = const.tile([P, P], F32)
            make_identity(nc, ident)
            mask_s = const.tile([P, P], F32)
            nc.sync.dma_start(mask_s[:], mask[:])
            eps_t = const.tile([P, 1], F32)
            nc.vector.memset(eps_t[:], EPS)
            vt_t = const.tile([1, BUILD_VER], F32)
            nc.sync.dma_start(vt_t[:], vtag[:])

            # ---- embedding: x = x0 + pe ----
            # per-chunk tiles so Tile's dependency tracking stays fine-grained:
            # readers of chunk t must not wait on writers of other chunks
            xs = [xpool.tile([P, D], F32, tag=f"x{t}", name=f"x{t}") for t in range(TC)]
            for t in range(TC):
                nc.sync.dma_start(xs[t][:], x0_t[:, t, :])
                pe_c = pet.tile([P, D], F32)
                nc.sync.dma_start(pe_c[:], pe_t[:, t, :])
                nc.vector.tensor_add(xs[t][:], xs[t][:], pe_c[:])

            def transpose_x(dst_xt):
                """PE-transpose x_nat [t,d] -> dst_xt [128, KC, S_c] ([d,t])."""
                for t in range(TC):
                    for k in range(KC):
                        pt = pstr.tile([P, P], F32)
                        nc.tensor.transpose(pt[:], xs[t][:, k * P:(k + 1) * P], ident[:])
                        nc.any.tensor_copy(dst_xt[:, k, t * P:(t + 1) * P], pt[:])

            def layernorm_chunk(xc, s_b, b_b, eng):
                """In-place LN over free dim (768) of xc [128, 768]."""
                stats = stp.tile([P, 3, 6], F32)
                for g in range(3):
                    nc.vector.bn_stats(stats[:, g, :], xc[:, g * 256:(g + 1) * 256])
                mv = stp.tile([P, 2], F32)
                nc.vector.bn_aggr(mv[:], stats[:])
                sd = stp.tile([P, 1], F32)
                nc.scalar.activation(sd[:], mv[:, 1:2], AF.Sqrt, bias=eps_t[:], scale=1.0)
                rs = stp.tile([P, 1], F32)
                nc.vector.reciprocal(rs[:], sd[:])
                nm = stp.tile([P, 1], F32)
                nc.vector.tensor_mul(nm[:], mv[:, 0:1], rs[:])
                nc.vector.tensor_scalar_mul(nm[:], nm[:], -1.0)
                nc.scalar.activation(xc, xc, AF.Identity, bias=nm[:], scale=rs[:])
                nc.vector.tensor_mul(xc, xc, s_b)
                eng.tensor_add(xc, xc, b_b)

            def dbg_dump(dst, idx, tiles):
                d_t = dst.rearrange("l (tc p) d -> l p tc d", p=P)
                for t in range(TC):
                    nc.sync.dma_start(d_t[idx, :, t, :], tiles[t][:])

            if dbg:
                dbg_dump(dbg_x, 0, xs)

            for l in range(L):
                # per-layer params (partition-broadcast replicas / per-o tiles)
                s1b = lnp.tile([P, D], F32, tag="s1b")
                b1b = lnp.tile([P, D], F32, tag="b1b")
                s2b = lnp.tile([P, D], F32, tag="s2b")
                b2b = lnp.tile([P, D], F32, tag="b2b")
                bvb = lnp.tile([P, D], F32, tag="bvb")
                blb = lnp.tile([P, D], F32, tag="blb")
                for t_, src in ((s1b, s1), (b1b, b1), (s2b, s2), (b2b, b2), (bvb, bv), (blb, bl)):
                    nc.gpsimd.dma_start(t_[:], _bcast(src[l]))
                bq_t = biasp.tile([P, NPAIR], F32, tag="bq")
                bk_t = biasp.tile([P, NPAIR], F32, tag="bk")
                nc.sync.dma_start(bq_t[:], bq[l].rearrange("(c p) -> p c", p=P))
                nc.sync.dma_start(bk_t[:], bk[l].rearrange("(c p) -> p c", p=P))

                x_T = xtpool.tile([P, KC, S_c], F32R, tag="xT")
                transpose_x(x_T)

                # ---- v projection (all heads at once, N>=256) ----
                wv_s = wfull.tile([P, KC, D], F32R, tag="wbig")
                nc.sync.dma_start(wv_s[:], wvT[l].rearrange("(k p) o -> p k o", p=P))
                v_aug = vap.tile([P, TC, NHEAD, DH + 1], BF16)
                nc.vector.memset(v_aug[:, :, :, DH:DH + 1], 1.0)
                for t in range(TC):
                    for os_, ow in ((0, 512), (512, 256)):
                        pv = ps512.tile([P, 512], F32, tag="ps512")
                        for k in range(KC):
                            nc.tensor.matmul(pv[:, :ow], x_T[:, k, t * P:(t + 1) * P],
                                             wv_s[:, k, os_:os_ + ow],
                                             start=(k == 0), stop=(k == KC - 1))
                        nh0 = os_ // DH
                        nc.vector.tensor_tensor(
                            v_aug[:, t, nh0:nh0 + ow // DH, 0:DH],
                            pv[:, :ow].rearrange("p (h d) -> p h d", d=DH),
                            bvb[:, os_:os_ + ow].rearrange("p (h d) -> p h d", d=DH),
                            ALU.add)

                attn_nat = [anpool.tile([P, D], F32, tag=f"an{t}", name=f"an{t}") for t in range(TC)]

                for pr in range(NPAIR):
                    wq_s = wqk.tile([P, KC, P], F32R, tag="wqk")
                    wk_s = wqk.tile([P, KC, P], F32R, tag="wqk")
                    nc.sync.dma_start(wq_s[:], wqT[l].rearrange("(k p) o -> p k o", p=P)[:, :, pr * P:(pr + 1) * P])
                    nc.sync.dma_start(wk_s[:], wkT[l].rearrange("(k p) o -> p k o", p=P)[:, :, pr * P:(pr + 1) * P])
                    qT_p = qkp.tile([P, S_c], BF16, tag="qk")
                    kT_p = qkp.tile([P, S_c], BF16, tag="qk")
                    for dst, w_s, b_t in ((qT_p, wq_s, bq_t), (kT_p, wk_s, bk_t)):
                        for nt in range(0, S_c, 512):
                            nw = min(512, S_c - nt)
                            pq = ps512.tile([P, 512], F32, tag="ps512")
                            for k in range(KC):
                                nc.tensor.matmul(pq[:, :nw], w_s[:, k, :],
                                                 x_T[:, k, nt:nt + nw],
                                                 start=(k == 0), stop=(k == KC - 1))
                            nc.scalar.activation(dst[:, nt:nt + nw], pq[:, :nw],
                                                 AF.Identity, bias=b_t[:, pr:pr + 1], scale=1.0)

                    for hh in range(2):
                        h = 2 * pr + hh
                        hs = DH * hh
                        # scores (transposed [j, i]) -> exp -> P_T (bf16)
                        p_t = ptp.tile([P, PTW], BF16, tag="pt")
                        for jc in range(TC):
                            w_ = W[jc]
                            i0 = P * jc
                            for so in range(0, w_, 512):
                                sw = min(512, w_ - so)
                                ps = ps512.tile([P, 512], F32, tag="ps512")
                                nc.tensor.matmul(
                                    ps[:, :sw],
                                    kT_p[hs:hs + DH, i0:i0 + P],
                                    qT_p[hs:hs + DH, i0 + so:i0 + so + sw],
                                    start=True, stop=True)
                                if so == 0:
                                    nc.vector.tensor_tensor(ps[:, :P], ps[:, :P], mask_s[:], ALU.add)
                                nc.scalar.activation(p_t[:, OFF[jc] + so:OFF[jc] + so + sw],
                                                     ps[:, :sw], AF.Exp, scale=0.125)
                        # P.V transposed: attnT_aug [65, i] per 512-wide i-tile
                        # (row 64 = softmax denominator), then PE-transpose back
                        # to natural [i, 65] and normalize.
                        for it in range((S_c + 511) // 512):
                            i_lo = 512 * it
                            i_hi = min(512 * (it + 1), S_c)
                            jcmax = min(TC - 1, (i_hi - 1) // P)
                            pat = psatt.tile([P, 512], F32, tag="pat")
                            for jc in range(jcmax + 1):
                                s0 = max(i_lo, P * jc)
                                w_ = i_hi - s0
                                o_ = OFF[jc] + s0 - P * jc
                                nc.tensor.matmul(
                                    pat[0:DH + 1, s0 - i_lo:s0 - i_lo + w_],
                                    v_aug[:, jc, h, :],
                                    p_t[:, o_:o_ + w_],
                                    start=(jc == 0), stop=(jc == jcmax))
                            atT = ztp.tile([DH + 1, 512], F32, tag="atT")
                            nc.any.tensor_copy(atT[:, :i_hi - i_lo], pat[0:DH + 1, :i_hi - i_lo])
                            for k_ in range((i_hi - i_lo) // P):
                                ic = 4 * it + k_
                                pa2 = pstr.tile([P, P], F32, tag="pt")
                                nc.tensor.transpose(pa2[:, 0:DH + 1], atT[:, k_ * P:(k_ + 1) * P],
                                                    ident[0:DH + 1, 0:DH + 1])
                                r_ = stp.tile([P, 1], F32)
                                nc.vector.reciprocal(r_[:], pa2[:, DH:DH + 1])
                                nc.vector.tensor_mul(attn_nat[ic][:, h * DH:(h + 1) * DH],
                                                     pa2[:, 0:DH],
                                                     r_[:].to_broadcast((P, DH)))

                if dbg:
                    dbg_dump(dbg_attn, l, attn_nat)

                # ---- residual + LN1 ----
                for t in range(TC):
                    eng = nc.gpsimd
                    eng.tensor_add(xs[t][:], xs[t][:], attn_nat[t][:])
                    layernorm_chunk(xs[t][:], s1b[:], b1b[:], eng)

                # ---- linear + residual + LN2 ----
                x1_T = xtpool.tile([P, KC, S_c], F32R, tag="xT")
                transpose_x(x1_T)
                wl_s = wfull.tile([P, KC, D], F32R, tag="wbig")
                nc.sync.dma_start(wl_s[:], wlT[l].rearrange("(k p) o -> p k o", p=P))
                for t in range(TC):
                    zt = ztp.tile([P, D], F32, tag="zt")
                    for os_, ow in ((0, 512), (512, 256)):
                        pl_ = ps512.tile([P, 512], F32, tag="ps512")
                        for k in range(KC):
                            nc.tensor.matmul(pl_[:, :ow], x1_T[:, k, t * P:(t + 1) * P],
                                             wl_s[:, k, os_:os_ + ow],
                                             start=(k == 0), stop=(k == KC - 1))
                        nc.vector.tensor_tensor(zt[:, os_:os_ + ow], pl_[:, :ow],
                                                blb[:, os_:os_ + ow], ALU.add)
                    eng = nc.gpsimd
                    eng.tensor_add(xs[t][:], xs[t][:], zt[:])
                    layernorm_chunk(xs[t][:], s2b[:], b2b[:], eng)

                if dbg:
                    dbg_dump(dbg_x, l + 1, xs)

            # ---- unembed ----
            x4_T = xtpool.tile([P, KC, S_c], BF16, tag="xT")
            transpose_x(x4_T)
            lctx.close()  # free layer-phase SBUF/PSUM before unembed pools
            psu = ctx.enter_context(tc_.tile_pool(name="psu", bufs=6, space="PSUM"))
            upool = ctx.enter_context(tc_.tile_pool(name="upool", bufs=4))
            ubp = ctx.enter_context(tc_.tile_pool(name="ubp", bufs=2))
            lop = ctx.enter_context(tc_.tile_pool(name="lop", bufs=3))
            uT_t = uT.rearrange("(k p) v -> p k v", p=P)
            for vs in range(0, VSH_c, 512):
                vw = min(512, VSH_c - vs)
                u_s = upool.tile([P, KC, 512], BF16, tag="u")
                nc.sync.dma_start(u_s[:, :, :vw], uT_t[:, :, vs:vs + vw])
                ub_b = ubp.tile([P, 512], F32, tag="ubb")
                nc.gpsimd.dma_start(ub_b[:, :vw], _bcast(ub[vs:vs + vw]))
                for t in range(TC):
                    pu = psu.tile([P, 512], F32, tag="psu")
                    for k in range(KC):
                        nc.tensor.matmul(pu[:, :vw], x4_T[:, k, t * P:(t + 1) * P],
                                         u_s[:, k, :vw],
                                         start=(k == 0), stop=(k == KC - 1))
                    lo = lop.tile([P, 512], F32, tag="lo")
                    nc.vector.tensor_tensor(lo[:, :vw], pu[:, :vw], ub_b[:, :vw], ALU.add)
                    nc.sync.dma_start(logits_t[:, t, vs:vs + vw], lo[:, :vw])

    nc.compile()
    return nc


_CACHE = {}


def get_program(S_c=S, L=4, VSH_c=VSH, dbg=False):
    key = (S_c, L, VSH_c, dbg)
    if key not in _CACHE:
        _CACHE[key] = build_program(S_c, L, VSH_c, dbg)
    return _CACHE[key]


def make_mask():
    jl = np.arange(P)[:, None]
    il = np.arange(P)[None, :]
    return np.where(jl <= il, 0.0, NEG).astype(np.float32)


def make_core_inputs(tokens, embed, pe, wq_w, wq_b, wk_w, wk_b, wv_w, wv_b,
                     lin_w, lin_b, n1_s, n1_b, n2_s, n2_b, unembed_w, unembed_b,
                     S_c=S, L=4, VSH_c=VSH, n_vshard=4):
    """Host-side sharding: returns list of in_maps (one per core)."""
    c = np.ascontiguousarray
    f = np.float32
    tokens = np.asarray(tokens)
    embed = np.asarray(embed, f)
    pe_s = c(np.asarray(pe, f)[:S_c])
    wqT = c(np.asarray(wq_w, f)[:L].transpose(0, 2, 1))
    wkT = c(np.asarray(wk_w, f)[:L].transpose(0, 2, 1))
    wvT = c(np.asarray(wv_w, f)[:L].transpose(0, 2, 1))
    wlT = c(np.asarray(lin_w, f)[:L].transpose(0, 2, 1))
    upad = np.zeros((n_vshard * VSH_c, D), f)
    ubpad = np.zeros((n_vshard * VSH_c,), f)
    nv = min(VOCAB, n_vshard * VSH_c, np.asarray(unembed_w).shape[0])
    upad[:nv] = np.asarray(unembed_w, f)[:nv]
    ubpad[:nv] = np.asarray(unembed_b, f)[:nv]
    mask = make_mask()
    common = dict(vtag=np.zeros((1, BUILD_VER), f), pe=pe_s, wqT=wqT, wkT=wkT, wvT=wvT, wlT=wlT,
                  bq=c(np.asarray(wq_b, f)[:L]), bk=c(np.asarray(wk_b, f)[:L]),
                  bv=c(np.asarray(wv_b, f)[:L]), bl=c(np.asarray(lin_b, f)[:L]),
                  s1=c(np.asarray(n1_s, f)[:L]), b1=c(np.asarray(n1_b, f)[:L]),
                  s2=c(np.asarray(n2_s, f)[:L]), b2=c(np.asarray(n2_b, f)[:L]),
                  mask=mask)
    n_batch_groups = NCORES // n_vshard
    in_maps = []
    for core in range(NCORES):
        b = core // n_vshard
        s_ = core % n_vshard
        x0 = c(embed[tokens[b, :S_c]])
        import ml_dtypes
        uT_c = c(upad[s_ * VSH_c:(s_ + 1) * VSH_c].T.astype(ml_dtypes.bfloat16))
        in_maps.append(dict(common, x0=x0, uT=uT_c,
                            ub=c(ubpad[s_ * VSH_c:(s_ + 1) * VSH_c])))
    return in_maps


def kernel(**inputs):
    nc = get_program(S, 4, VSH, dbg=False)
    in_maps = make_core_inputs(**inputs)
    res = run_bass_kernel_spmd(nc, in_maps, core_ids=list(range(NCORES)))
    out = np.zeros((B, S, VOCAB), np.float32)
    for core in range(NCORES):
        b = core // 4
        s_ = core % 4
        lo = res.results[core]["logits"]
        v0 = s_ * VSH
        v1 = min(v0 + VSH, VOCAB)
        if v1 > v0:
            out[b, :, v0:v1] = lo[:, :v1 - v0]
    return out
